# revision 55
# baseline (speedup 1.0000x reference)
"""Trainium2 Bass kernel for Convpass-swin hypernet fused adapter.

Reference computation (per batch sample):
  h      = relu(x @ Wm1 + bm1)                    # [B,H,W,64]
  prompt = mean_hw(h) @ Wm2 + bm2                 # [B,64]  (mean commutes with matmul)
  wflat  = (emb + prompt) @ Wh + bh               # [B,96*96*9]
  xd     = quickgelu(x @ Wd + bd)                 # [B,H,W,96]
  y      = quickgelu(conv3x3(xd, wflat))          # per-sample dynamic grouped conv
  out    = y @ Wu + bu                            # [B,H,W,384]

Sharding: data-parallel over batch B=64 across 8 cores (8 samples/core),
weights replicated.

Structure (measured: 62860 ns TimelineSim, rel err 1.68e-2 vs 2e-2 gate;
baseline was 78070 ns):
  * Conv runs in fp8 with DoubleRow matmuls (0.5 cycles/row): the
    stationary pairs (w_hi, w_lo) where w_hi = fp8(S*w0) is a
    host-precomputed constant shared by all samples and
    w_lo = fp8(S*w0 - w_hi + S*delta_b) is the per-sample correction
    produced directly by the hypernet stream's scalar_tensor_tensor.
    w8a layout [i, tap, slot, o] nests slots INSIDE taps: stream writes
    are chunk-local and conv reads tap-local (no false range deps) while
    the DoubleRow pair stride (1+b)*96 stays 16-byte aligned.
    The moving operand is a broadcast (stride-0) pair of one contiguous
    420-element window of the padded fp8 xd buffer (junk columns land in
    unused psum positions, skipped by the strided gelu read).
  * Error budget: taps 7-8 also accumulate the fp8 residual
    xd_lo = fp8(xd16 - xd8) via a second DoubleRow matmul, trimming the
    dominant xd quantization noise; the prompt stays fp16. The whole
    pipeline is deterministic, so the measured error is exact.
  * Engine balance: meta relu+accum on DVE (keeps Act free so the
    px-psum rotation never paces the PE), xd16->xd8 copy and the
    residual subtract on the otherwise-idle GPSIMD, psum->sbuf output
    copies split Act/DVE. The w0 residual seed rides the hypernet
    matmul as a 65th contraction row (ones-row in pvec), so the
    stream's w_lo write is a 1-input scaled copy that ALTERNATES
    between Act and DVE per chunk - consecutive stream->conv
    dependency links ride independent queues, halving the serial
    chain the Tile counting semaphores would otherwise create.
  * bu folds into the output projection as an extra contraction row
    against a constant ones-row in the y tensor, so the 48 psum->sbuf
    copies are pure dtype copies.
  * Schedule: meta+xd interleave at the x-DMA pace; the Wh stream
    starts right after the prompt and is consumed at the chunk-DMA
    pace; the conv+out tail runs on two 3-tile psum rings.
"""
import numpy as np

import concourse.bass as bass
import concourse.tile as tile
import concourse.mybir as mybir
from concourse import bacc
from concourse.bass_utils import run_bass_kernel_spmd

F32 = mybir.dt.float32
F16 = mybir.dt.float16
F8 = mybir.dt.float8e4
AF = mybir.ActivationFunctionType
AX = mybir.AxisListType
DR = mybir.MatmulPerfMode.DoubleRow

# problem constants
B, H, W, C = 64, 28, 28, 384
DIM, E, KK = 96, 64, 3
NCORES = 8
BL = B // NCORES          # samples per core
P = H * W                 # 784 positions per sample
HP = H + 2                # padded spatial
WH_COLS = DIM * DIM * 9   # 82944
NCH = 18                  # Wh stream chunks
CHW = WH_COLS // NCH      # 4608 columns per chunk
GRP = CHW // DIM          # 48 (t,o) groups per chunk
WH_SCALE = 256.0          # fp8 range scaling for Wh
W_SCALE = 64.0            # fp8 scale on conv weights (w_hi/w_lo slots)
HF = P // 2               # 392 positions per half
HPR = HP + 1              # padded rows incl. one spare (window overrun)
CW = 14 * HP              # 420: conv psum window length (junk cols unused)
TO = 9 * DIM              # 864 (t,o) pairs
NSLOT = 1 + BL            # w8 slots: [hi, lo(b=0..7)]
EXACT_TAPS = (7, 8)       # taps that also accumulate the xd_lo residual
NRESC = 6                 # tail psum ring tiles
NRES_ACTIVE = 0           # resident conv accumulators riding the stream


def build_nc():
    nc = bacc.Bacc("TRN2", target_bir_lowering=False, debug=False)

    xt_d = nc.dram_tensor("xt", [128, 3, BL, P], F16, kind="ExternalInput").ap()
    w16_d = nc.dram_tensor("w16", [128, 864], F16, kind="ExternalInput").ap()
    bias_d = nc.dram_tensor("bias", [128, 66], F32, kind="ExternalInput").ap()
    wh_d = nc.dram_tensor("wh", [E + 1, WH_COLS], F8, kind="ExternalInput").ap()
    whi_d = nc.dram_tensor("whi", [DIM, TO], F8, kind="ExternalInput").ap()
    one_d = nc.dram_tensor("one", [1, BL * P], F16, kind="ExternalInput").ap()
    yo_d = nc.dram_tensor("yo", [128, 3, BL, P], F16, kind="ExternalOutput").ap()

    with tile.TileContext(nc) as tc:
        with (
            tc.tile_pool(name="const", bufs=1) as cp,
            tc.tile_pool(name="persist", bufs=1) as pp,
            tc.tile_pool(name="scr", bufs=3) as sp,
            tc.tile_pool(name="wh", bufs=18) as wh_p,
            tc.tile_pool(name="ob", bufs=6) as ob_p,
        ):
            # ---- constants (meta/xd weights first), then the x stream ----
            w16_sb = cp.tile([128, 864], F16)
            nc.sync.dma_start(w16_sb[:, 0:192], w16_d[:, 0:192])

            xt_sb = pp.tile([128, 3, BL, P], F16)
            # first sample in halves so meta(0,h0) starts as early as possible
            nc.sync.dma_start(xt_sb[:, :, 0, 0:HF], xt_d[:, :, 0, 0:HF])
            nc.sync.dma_start(w16_sb[:, 192:480], w16_d[:, 192:480])
            nc.sync.dma_start(xt_sb[:, :, 0, HF:P], xt_d[:, :, 0, HF:P])
            nc.sync.dma_start(xt_sb[:, :, 1, :], xt_d[:, :, 1, :])
            bias_sb = cp.tile([128, 66], F32)
            nc.sync.dma_start(bias_sb[:], bias_d[:])
            for b in range(2, BL):
                nc.sync.dma_start(xt_sb[:, :, b, :], xt_d[:, :, b, :])
            # w8a: [i, tap, slot, o] (slot 0 = w_hi const). Nesting slots
            # INSIDE taps makes the stream's writes and the conv's reads
            # tap-local (no false deps) while keeping the DoubleRow pair
            # stride (1+b)*96 16-aligned.
            w8a = pp.tile([DIM, 9, NSLOT, DIM], F8)
            nc.sync.dma_start(
                w8a[:, :, 0, :], whi_d[:].rearrange("i (t o) -> i t o", t=9))
            nc.sync.dma_start(w16_sb[:, 480:864], w16_d[:, 480:864])
            # y buffer with a constant ones-row folded into out-proj for bu
            y97 = pp.tile([97, BL, P], F16)
            nc.sync.dma_start(y97[96:97, :, :], one_d[:].rearrange("o (b p) -> o b p", b=BL))

            # ---- persistent state ----
            xd16 = pp.tile([DIM, BL, P], F16)             # fp16 gelu(x@Wd+bd)
            xd8 = pp.tile([DIM, BL, HPR, HP], F8)         # padded fp8 xd
            xdl = pp.tile([DIM, BL, HPR, HP], F8)         # padded fp8 residual
            hsum = pp.tile([E, BL, 2], F32)               # per-(sample,half) relu sums
            hbar = pp.tile([E, BL], F32)
            pvec = pp.tile([E + 1, BL], F16)              # prompt_raw + ones row
            nc.gpsimd.memset(pvec[E:E + 1, :], 1.0)
            zeros = cp.tile([E, 1], F32)
            nc.gpsimd.memset(zeros[:], 0.0)

            # zero the conv halo rings (and window-overrun row) on Pool
            for buf in (xd8, xdl):
                nc.gpsimd.memset(buf[:, :, 0:1, :], 0.0)
                nc.gpsimd.memset(buf[:, :, HP - 1:HPR, :], 0.0)
                nc.gpsimd.memset(buf[:, :, 1:HP - 1, 0:1], 0.0)
                nc.gpsimd.memset(buf[:, :, 1:HP - 1, HP - 1:HP], 0.0)

            ps2 = tc.tile_pool(name="ps2", bufs=1, space="PSUM")
            p2 = ps2.__enter__()

            # ---- phase 1: meta + xd interleaved at the x-DMA pace ----
            ps1 = tc.tile_pool(name="ps1", bufs=1, space="PSUM")
            p1 = ps1.__enter__()

            def emit_meta(b):
                for h2 in range(2):
                    ph = p1.tile([E, HF], F32, name="ph", tag="ph", bufs=2)
                    for c in range(3):
                        nc.tensor.matmul(ph[:], w16_sb[:, c * 64:(c + 1) * 64],
                                         xt_sb[:, c, b, h2 * HF:(h2 + 1) * HF],
                                         start=(c == 0), stop=(c == 2))
                    h_scr = sp.tile([E, HF], F32, tag="hscr")
                    # relu+accum on DVE: keeps Act free to run the xd gelus
                    # ahead of the PE (px-buffer rotation would otherwise
                    # pace the whole stream at Act speed)
                    nc.vector.scalar_tensor_tensor(
                        h_scr[:], ph[:], bias_sb[0:E, 0:1],
                        zeros[:].broadcast_to([E, HF]),
                        op0=mybir.AluOpType.add,
                        op1=mybir.AluOpType.max,
                        accum_out=hsum[:, b, h2:h2 + 1])

            def emit_xd_half(b, h2):
                px = p1.tile([DIM, HF], F32, name="px", tag="px", bufs=3)
                for c in range(3):
                    nc.tensor.matmul(px[:], w16_sb[:, 192 + c * 96:192 + (c + 1) * 96],
                                     xt_sb[:, c, b, h2 * HF:(h2 + 1) * HF],
                                     start=(c == 0), stop=(c == 2))
                nc.scalar.activation(
                    xd16[:, b, h2 * HF:(h2 + 1) * HF], px[:],
                    AF.Gelu_apprx_sigmoid, bias=bias_sb[0:DIM, 1:2])
                x16v = xd16[:, b, h2 * HF:(h2 + 1) * HF] \
                    .rearrange("p (r c) -> p r c", r=14)
                d8 = xd8[:, b, 1 + h2 * 14:15 + h2 * 14, 1:29]
                nc.gpsimd.tensor_copy(d8, x16v)
                # residual: xd16 - xd8 (on the otherwise-idle GPSIMD)
                nc.gpsimd.tensor_sub(
                    xdl[:, b, 1 + h2 * 14:15 + h2 * 14, 1:29], x16v, d8)

            # meta is x-DMA-paced; fill the PE slack with early samples' xd
            xd_queue = [(b, h2) for b in range(BL) for h2 in range(2)]
            xq = 0
            for b in range(BL):
                emit_meta(b)
                # one xd half per sample keeps PE just under the DMA pace
                if b >= 1 and xq < len(xd_queue):
                    xb, xh = xd_queue[xq]
                    if xb < b:   # its x is already resident
                        emit_xd_half(xb, xh)
                        xq += 1
            # prompt chain
            nc.vector.reduce_sum(hbar[:], hsum[:], axis=AX.X)
            ppm = p1.tile([E, BL], F32, name="ppm", tag="ppm", bufs=1)
            nc.tensor.matmul(ppm[:], bias_sb[0:E, 2:66], hbar[:],
                             start=True, stop=True)
            nc.scalar.activation(pvec[0:E, :], ppm[:], AF.Copy)

            # ---- hypernet stream: w_lo slots = fp8(w0r + delta*W_SCALE) ----
            def emit_ph3(jd):
                whc = wh_p.tile([E + 1, CHW], F8, tag="whc")
                nc.sync.dma_start(whc[:], wh_d[:, jd * CHW:(jd + 1) * CHW])
                pwg = p2.tile([DIM, GRP * BL], F32, name="pwg", tag="pwg", bufs=2)
                for gg in range(GRP):
                    nc.tensor.matmul(pwg[:, gg * BL:(gg + 1) * BL],
                                     whc[:, gg * DIM:(gg + 1) * DIM], pvec[:],
                                     start=True, stop=True)
                t, o0 = jd // 2, (jd % 2) * GRP
                dst = w8a[:, t, 1:NSLOT, o0:o0 + GRP]
                srcv = pwg[:].rearrange("i (g b) -> i b g", g=GRP)
                # alternate copy engines so consecutive stream->conv links
                # ride independent queues (halves the serial chain)
                if jd % 2 == 0:
                    nc.scalar.activation(dst, srcv, AF.Copy,
                                         scale=W_SCALE / WH_SCALE)
                else:
                    nc.vector.tensor_scalar_mul(dst, srcv, W_SCALE / WH_SCALE)

            # ---- resident conv: DoubleRow taps accumulate while Wh streams ----
            RES_PAIRS = [(0, 0), (0, 1), (1, 0), (1, 1), (2, 0), (2, 1)][:NRES_ACTIVE]
            res_box = []

            def open_res():
                pool = tc.tile_pool(name="res", bufs=1, space="PSUM")
                pr = pool.__enter__()
                res_box.append(pool)
                return [pr.tile([128, CW], F32, name=f"pyr{i}")
                        for i in range(NRESC)]

            py_res = []

            def conv_mm(py, b, h2, t, first, last):
                dy, dx = t // 3, t % 3
                # stationary pairs (w_hi const, w_lo sample b): slots {0, 1+b}
                base_w = w8a[:, t, 0, :]
                lhsT = bass.AP(base_w.tensor, base_w.offset,
                               [list(base_w.ap[0]), [(1 + b) * DIM, 2],
                                [1, DIM]])
                base = (h2 * 14 + dy) * HP + dx
                srcs = [xd8]
                if t in EXACT_TAPS:
                    srcs.append(xdl)
                for k, srcbuf in enumerate(srcs):
                    flat = srcbuf[:, b, :, :].rearrange("i r c -> i (r c)")
                    rhs = flat[:, base:base + CW] \
                        .rearrange("i (o m) -> i o m", o=1).broadcast_to([DIM, 2, CW])
                    nc.tensor.matmul(py[0:DIM, :], lhsT, rhs,
                                     start=(first and k == 0),
                                     stop=(last and k == len(srcs) - 1),
                                     perf_mode=DR)

            def emit_res_tap(t):
                for i, (b, h2) in enumerate(RES_PAIRS):
                    conv_mm(py_res[i], b, h2, t, t == 0, t == 8)

            tap_next = 0

            def emit_ready_taps(jc):
                nonlocal tap_next
                while tap_next < 9 and 2 * tap_next + 1 <= jc and py_res:
                    emit_res_tap(tap_next)
                    tap_next += 1

            # stream loop: early chunks drain the remaining xd halves
            # (1-2 per chunk); once xd is done the ps1 psum pool is released
            # to the resident conv accumulators and taps ride the stream.
            jd = 0
            while jd < NCH:
                emit_ph3(jd)
                ndrain = 2 if jd % 2 == 0 else 1
                while ndrain > 0 and xq < len(xd_queue):
                    b2, h22 = xd_queue[xq]
                    emit_xd_half(b2, h22)
                    xq += 1
                    ndrain -= 1
                    if xq == len(xd_queue):
                        ps1.__exit__(None, None, None)
                        py_res.extend(open_res())
                if xq >= len(xd_queue):
                    emit_ready_taps(jd - 2)
                jd += 1
            if xq < len(xd_queue):
                while xq < len(xd_queue):
                    b2, h22 = xd_queue[xq]
                    emit_xd_half(b2, h22)
                    xq += 1
                ps1.__exit__(None, None, None)
                py_res.extend(open_res())
            emit_ready_taps(NCH - 1)

            def emit_gelu(py, b, h2):
                nc.scalar.activation(
                    y97[0:DIM, b, h2 * HF:(h2 + 1) * HF],
                    py[0:DIM, :].rearrange("p (r c) -> p r c", r=14, c=30)[:, :, 0:28],
                    AF.Gelu_apprx_sigmoid, scale=1.0 / W_SCALE)

            for i, (b, h2) in enumerate(RES_PAIRS):
                emit_gelu(py_res[i], b, h2)

            # ---- tail: remaining convs + output projection for all samples ----
            # conv halves and out matmuls share one 6-tile psum ring so
            # write-after-read waits always point several pipeline stages back
            rot = 0
            conv_ring = [0]
            out_ring = [0]

            def next_conv_psum():
                py = py_res[conv_ring[0] % 3]
                conv_ring[0] += 1
                return py

            def next_out_psum():
                py = py_res[3 + out_ring[0] % 3]
                out_ring[0] += 1
                return py

            def emit_out_half(b, ob, h2):
                nonlocal rot
                for c in range(3):
                    po = next_out_psum()
                    nc.tensor.matmul(po[:, 0:HF], w16_sb[0:97, 480 + c * 128:480 + (c + 1) * 128],
                                     y97[:, b, h2 * HF:(h2 + 1) * HF],
                                     start=True, stop=True)
                    dst = ob[:, c, h2, :]
                    if rot % 2 == 1:
                        nc.scalar.activation(dst, po[:, 0:HF], AF.Copy)
                    else:
                        nc.vector.tensor_copy(dst, po[:, 0:HF])
                    rot += 1
                nc.sync.dma_start(yo_d[:, :, b, h2 * HF:(h2 + 1) * HF],
                                  ob[:, :, h2, :])

            def emit_conv_half(b, h2):
                py = next_conv_psum()
                for t in range(9):
                    conv_mm(py, b, h2, t, t == 0, t == 8)
                emit_gelu(py, b, h2)

            conv_halves = [p for p in
                           [(b, h2) for b in range(BL) for h2 in range(2)]
                           if p not in RES_PAIRS]
            out_halves = [(b, h2) for b in range(BL) for h2 in range(2)]
            oq = 0
            obs = {}
            ready = [NRES_ACTIVE + i for i in range(len(conv_halves))]

            def emit_next_out():
                nonlocal oq
                b, h2 = out_halves[oq]
                if h2 == 0:
                    obs[b] = ob_p.tile([128, 3, 2, HF], F16, name=f"ob{b}", tag="ob")
                emit_out_half(b, obs[b], h2)
                oq += 1

            for i, (b, h2) in enumerate(conv_halves):
                emit_conv_half(b, h2)
                want = min(ready[i], len(out_halves))
                while oq < want and oq < (16 * (i + 1)) // len(conv_halves) + 4:
                    emit_next_out()
            while oq < len(out_halves):
                emit_next_out()
            res_box[0].__exit__(None, None, None)
            ps2.__exit__(None, None, None)

    nc.compile()
    return nc


_NC_CACHE = None


def _get_nc():
    global _NC_CACHE
    if _NC_CACHE is None:
        _NC_CACHE = build_nc()
    return _NC_CACHE


def _prep_inputs(x, Wd, bd, Wm1, bm1, Wm2, bm2, Wh, bh, emb, Wu, bu):
    """Host-side prep: pure layout/dtype transforms + x-independent weight
    folding (w0/w_hi/w0r rows, bias packing)."""
    import ml_dtypes
    f16 = np.float16
    f8 = ml_dtypes.float8_e4m3
    Wh = np.asarray(Wh, np.float32)
    w0 = (np.asarray(emb, np.float32) + np.asarray(bm2, np.float32)) @ Wh \
        + np.asarray(bh, np.float32)
    # (e, o, i, t) -> tap-major (e, t, o, i); fp8 with range scaling
    whp = Wh.reshape(E, DIM, DIM, KK * KK).transpose(0, 3, 1, 2).reshape(E, WH_COLS)
    wh8 = np.ascontiguousarray(whp * WH_SCALE).astype(f8)
    # w0 -> [i, (t,o)]; split into fp8 hi + fp16 residual (at W_SCALE)
    w0p = np.ascontiguousarray(
        w0.reshape(DIM, DIM, KK * KK).transpose(2, 0, 1).reshape(TO, DIM).T
    ).astype(np.float32) * W_SCALE
    whi = w0p.astype(f8)
    w0r = (w0p - whi.astype(np.float32))
    w0r_row = (w0r.T.reshape(1, WH_COLS) * (WH_SCALE / W_SCALE)).astype(f8)
    wh8 = np.concatenate([wh8, w0r_row], axis=0)

    w16 = np.zeros((128, 864), f16)
    w16[:, 0:192] = np.asarray(Wm1, np.float32).reshape(3, 128, E) \
        .transpose(1, 0, 2).reshape(128, 192)
    w16[:, 192:480] = np.asarray(Wd, np.float32).reshape(3, 128, DIM) \
        .transpose(1, 0, 2).reshape(128, 288)
    w16[0:DIM, 480:864] = np.asarray(Wu, np.float32)
    w16[DIM:97, 480:864] = np.asarray(bu, np.float32).reshape(1, 384)

    bias = np.zeros((128, 66), np.float32)
    bias[0:E, 0] = np.asarray(bm1, np.float32)
    bias[0:DIM, 1] = np.asarray(bd, np.float32)
    bias[0:E, 2:66] = np.asarray(Wm2, np.float32) / P   # pre-scaled fp32 Wm2

    ones = np.ones((1, BL * P), f16)

    shared = {"w16": w16, "bias": bias, "wh": wh8, "whi": whi,
              "one": ones}
    xs = np.asarray(x, np.float32).astype(f16).reshape(B, P, C)
    in_maps = []
    for k in range(NCORES):
        xt = xs[k * BL:(k + 1) * BL].reshape(BL, P, 3, 128).transpose(3, 2, 0, 1)
        m = dict(shared)
        m["xt"] = np.ascontiguousarray(xt)
        in_maps.append(m)
    return in_maps


def _run(inputs, **spmd_kwargs):
    nc = _get_nc()
    in_maps = _prep_inputs(**inputs)
    res = run_bass_kernel_spmd(nc, in_maps, core_ids=list(range(NCORES)), **spmd_kwargs)
    parts = []
    for r in res.results:
        yo = np.asarray(r["yo"])                      # [128, 3, BL, P] fp16
        parts.append(yo.transpose(2, 3, 1, 0).reshape(BL, P, C))
    out = np.concatenate(parts, 0).astype(np.float32)
    return out.reshape(B, H, W, C), res


def kernel(**inputs) -> np.ndarray:
    out, _ = _run(inputs)
    return out


# revision 56
# speedup vs baseline: 1.0154x; 1.0154x over previous
"""Trainium2 Bass kernel for Convpass-swin hypernet fused adapter.

Reference computation (per batch sample):
  h      = relu(x @ Wm1 + bm1)                    # [B,H,W,64]
  prompt = mean_hw(h) @ Wm2 + bm2                 # [B,64]  (mean commutes with matmul)
  wflat  = (emb + prompt) @ Wh + bh               # [B,96*96*9]
  xd     = quickgelu(x @ Wd + bd)                 # [B,H,W,96]
  y      = quickgelu(conv3x3(xd, wflat))          # per-sample dynamic grouped conv
  out    = y @ Wu + bu                            # [B,H,W,384]

Sharding: data-parallel over batch B=64 across 8 cores (8 samples/core),
weights replicated.

Structure (measured: 62860 ns TimelineSim, rel err 1.68e-2 vs 2e-2 gate;
baseline was 78070 ns):
  * Conv runs in fp8 with DoubleRow matmuls (0.5 cycles/row): the
    stationary pairs (w_hi, w_lo) where w_hi = fp8(S*w0) is a
    host-precomputed constant shared by all samples and
    w_lo = fp8(S*w0 - w_hi + S*delta_b) is the per-sample correction
    produced directly by the hypernet stream's scalar_tensor_tensor.
    w8a layout [i, tap, slot, o] nests slots INSIDE taps: stream writes
    are chunk-local and conv reads tap-local (no false range deps) while
    the DoubleRow pair stride (1+b)*96 stays 16-byte aligned.
    The moving operand is a broadcast (stride-0) pair of one contiguous
    420-element window of the padded fp8 xd buffer (junk columns land in
    unused psum positions, skipped by the strided gelu read).
  * Error budget: taps 7-8 also accumulate the fp8 residual
    xd_lo = fp8(xd16 - xd8) via a second DoubleRow matmul, trimming the
    dominant xd quantization noise; the prompt stays fp16. The whole
    pipeline is deterministic, so the measured error is exact.
  * Engine balance: meta relu+accum on DVE (keeps Act free so the
    px-psum rotation never paces the PE), xd16->xd8 copy and the
    residual subtract on the otherwise-idle GPSIMD, psum->sbuf output
    copies split Act/DVE. The w0 residual seed rides the hypernet
    matmul as a 65th contraction row (ones-row in pvec), so the
    stream's w_lo write is a 1-input scaled copy that ALTERNATES
    between Act and DVE per chunk - consecutive stream->conv
    dependency links ride independent queues, halving the serial
    chain the Tile counting semaphores would otherwise create.
  * bu folds into the output projection as an extra contraction row
    against a constant ones-row in the y tensor, so the 48 psum->sbuf
    copies are pure dtype copies.
  * Schedule: meta+xd interleave at the x-DMA pace; the Wh stream
    starts right after the prompt and is consumed at the chunk-DMA
    pace; the conv+out tail runs on two 3-tile psum rings.
"""
import numpy as np

import concourse.bass as bass
import concourse.tile as tile
import concourse.mybir as mybir
from concourse import bacc
from concourse.bass_utils import run_bass_kernel_spmd

F32 = mybir.dt.float32
F16 = mybir.dt.float16
F8 = mybir.dt.float8e4
AF = mybir.ActivationFunctionType
AX = mybir.AxisListType
DR = mybir.MatmulPerfMode.DoubleRow

# problem constants
B, H, W, C = 64, 28, 28, 384
DIM, E, KK = 96, 64, 3
NCORES = 8
BL = B // NCORES          # samples per core
P = H * W                 # 784 positions per sample
HP = H + 2                # padded spatial
WH_COLS = DIM * DIM * 9   # 82944
NCH = 18                  # Wh stream chunks
CHW = WH_COLS // NCH      # 4608 columns per chunk
GRP = CHW // DIM          # 48 (t,o) groups per chunk
WH_SCALE = 256.0          # fp8 range scaling for Wh
W_SCALE = 64.0            # fp8 scale on conv weights (w_hi/w_lo slots)
HF = P // 2               # 392 positions per half
HPR = HP + 1              # padded rows incl. one spare (window overrun)
CW = 14 * HP              # 420: conv psum window length (junk cols unused)
TO = 9 * DIM              # 864 (t,o) pairs
NSLOT = 1 + BL            # w8 slots: [hi, lo(b=0..7)]
EXACT_TAPS = (7, 8)       # taps that also accumulate the xd_lo residual
NRESC = 6                 # tail psum ring tiles
NRES_ACTIVE = 0           # resident conv accumulators riding the stream


def build_nc():
    nc = bacc.Bacc("TRN2", target_bir_lowering=False, debug=False)

    xt_d = nc.dram_tensor("xt", [128, 3, BL, P], F16, kind="ExternalInput").ap()
    w16_d = nc.dram_tensor("w16", [128, 864], F16, kind="ExternalInput").ap()
    bias_d = nc.dram_tensor("bias", [128, 66], F32, kind="ExternalInput").ap()
    wh_d = nc.dram_tensor("wh", [E + 1, WH_COLS], F8, kind="ExternalInput").ap()
    whi_d = nc.dram_tensor("whi", [DIM, TO], F8, kind="ExternalInput").ap()
    one_d = nc.dram_tensor("one", [1, BL * P], F16, kind="ExternalInput").ap()
    yo_d = nc.dram_tensor("yo", [128, 3, BL, P], F16, kind="ExternalOutput").ap()

    with tile.TileContext(nc) as tc:
        with (
            tc.tile_pool(name="const", bufs=1) as cp,
            tc.tile_pool(name="persist", bufs=1) as pp,
            tc.tile_pool(name="scr", bufs=3) as sp,
            tc.tile_pool(name="wh", bufs=18) as wh_p,
            tc.tile_pool(name="ob", bufs=6) as ob_p,
        ):
            # ---- constants (meta/xd weights first), then the x stream ----
            w16_sb = cp.tile([128, 864], F16)
            nc.sync.dma_start(w16_sb[:, 0:192], w16_d[:, 0:192])

            xt_sb = pp.tile([128, 3, BL, P], F16)
            # first sample in halves so meta(0,h0) starts as early as possible
            nc.sync.dma_start(xt_sb[:, :, 0, 0:HF], xt_d[:, :, 0, 0:HF])
            nc.sync.dma_start(w16_sb[:, 192:480], w16_d[:, 192:480])
            nc.sync.dma_start(xt_sb[:, :, 0, HF:P], xt_d[:, :, 0, HF:P])
            nc.sync.dma_start(xt_sb[:, :, 1, :], xt_d[:, :, 1, :])
            bias_sb = cp.tile([128, 66], F32)
            nc.sync.dma_start(bias_sb[:], bias_d[:])
            for b in range(2, BL):
                nc.sync.dma_start(xt_sb[:, :, b, :], xt_d[:, :, b, :])
            # w8a: [i, tap, slot, o] (slot 0 = w_hi const). Nesting slots
            # INSIDE taps makes the stream's writes and the conv's reads
            # tap-local (no false deps) while keeping the DoubleRow pair
            # stride (1+b)*96 16-aligned.
            w8a = pp.tile([DIM, 9, NSLOT, DIM], F8)
            nc.sync.dma_start(
                w8a[:, :, 0, :], whi_d[:].rearrange("i (t o) -> i t o", t=9))
            nc.sync.dma_start(w16_sb[:, 480:864], w16_d[:, 480:864])
            # y buffer with a constant ones-row folded into out-proj for bu
            y97 = pp.tile([97, BL, P], F16)
            nc.sync.dma_start(y97[96:97, :, :], one_d[:].rearrange("o (b p) -> o b p", b=BL))

            # ---- persistent state ----
            xd16 = pp.tile([DIM, BL, P], F16)             # fp16 gelu(x@Wd+bd)
            xd8 = pp.tile([DIM, BL, HPR, HP], F8)         # padded fp8 xd
            xdl = pp.tile([DIM, BL, HPR, HP], F8)         # padded fp8 residual
            hsum = pp.tile([E, BL, 2], F32)               # per-(sample,half) relu sums
            hbar = pp.tile([E, BL], F32)
            pvec = pp.tile([E + 1, BL], F16)              # prompt_raw + ones row
            nc.gpsimd.memset(pvec[E:E + 1, :], 1.0)
            zeros = cp.tile([E, 1], F32)
            nc.gpsimd.memset(zeros[:], 0.0)

            # zero the conv halo rings (and window-overrun row) on Pool
            for buf in (xd8, xdl):
                nc.gpsimd.memset(buf[:, :, 0:1, :], 0.0)
                nc.gpsimd.memset(buf[:, :, HP - 1:HPR, :], 0.0)
                nc.gpsimd.memset(buf[:, :, 1:HP - 1, 0:1], 0.0)
                nc.gpsimd.memset(buf[:, :, 1:HP - 1, HP - 1:HP], 0.0)

            ps2 = tc.tile_pool(name="ps2", bufs=1, space="PSUM")
            p2 = ps2.__enter__()

            # ---- phase 1: meta + xd interleaved at the x-DMA pace ----
            ps1 = tc.tile_pool(name="ps1", bufs=1, space="PSUM")
            p1 = ps1.__enter__()

            def emit_meta(b):
                for h2 in range(2):
                    ph = p1.tile([E, HF], F32, name="ph", tag="ph", bufs=2)
                    for c in range(3):
                        nc.tensor.matmul(ph[:], w16_sb[:, c * 64:(c + 1) * 64],
                                         xt_sb[:, c, b, h2 * HF:(h2 + 1) * HF],
                                         start=(c == 0), stop=(c == 2))
                    h_scr = sp.tile([E, HF], F32, tag="hscr")
                    # relu+accum on DVE: keeps Act free to run the xd gelus
                    # ahead of the PE (px-buffer rotation would otherwise
                    # pace the whole stream at Act speed)
                    nc.vector.scalar_tensor_tensor(
                        h_scr[:], ph[:], bias_sb[0:E, 0:1],
                        zeros[:].broadcast_to([E, HF]),
                        op0=mybir.AluOpType.add,
                        op1=mybir.AluOpType.max,
                        accum_out=hsum[:, b, h2:h2 + 1])

            def emit_xd_half(b, h2):
                px = p1.tile([DIM, HF], F32, name="px", tag="px", bufs=3)
                for c in range(3):
                    nc.tensor.matmul(px[:], w16_sb[:, 192 + c * 96:192 + (c + 1) * 96],
                                     xt_sb[:, c, b, h2 * HF:(h2 + 1) * HF],
                                     start=(c == 0), stop=(c == 2))
                nc.scalar.activation(
                    xd16[:, b, h2 * HF:(h2 + 1) * HF], px[:],
                    AF.Gelu_apprx_sigmoid, bias=bias_sb[0:DIM, 1:2])
                x16v = xd16[:, b, h2 * HF:(h2 + 1) * HF] \
                    .rearrange("p (r c) -> p r c", r=14)
                d8 = xd8[:, b, 1 + h2 * 14:15 + h2 * 14, 1:29]
                nc.gpsimd.tensor_copy(d8, x16v)
                # residual: xd16 - xd8 (on the otherwise-idle GPSIMD)
                nc.gpsimd.tensor_sub(
                    xdl[:, b, 1 + h2 * 14:15 + h2 * 14, 1:29], x16v, d8)

            # meta is x-DMA-paced; fill the PE slack with early samples' xd
            xd_queue = [(b, h2) for b in range(BL) for h2 in range(2)]
            xq = 0
            for b in range(BL):
                emit_meta(b)
                # one xd half per sample keeps PE just under the DMA pace
                if b >= 1 and xq < len(xd_queue):
                    xb, xh = xd_queue[xq]
                    if xb < b:   # its x is already resident
                        emit_xd_half(xb, xh)
                        xq += 1
            # prompt chain
            nc.vector.reduce_sum(hbar[:], hsum[:], axis=AX.X)
            ppm = p1.tile([E, BL], F32, name="ppm", tag="ppm", bufs=1)
            nc.tensor.matmul(ppm[:], bias_sb[0:E, 2:66], hbar[:],
                             start=True, stop=True)
            nc.scalar.activation(pvec[0:E, :], ppm[:], AF.Copy)

            # ---- hypernet stream: w_lo slots = fp8(w0r + delta*W_SCALE) ----
            def emit_ph3(jd):
                whc = wh_p.tile([E + 1, CHW], F8, tag="whc")
                nc.sync.dma_start(whc[:], wh_d[:, jd * CHW:(jd + 1) * CHW])
                pwg = p2.tile([DIM, GRP * BL], F32, name="pwg", tag="pwg", bufs=2)
                for gg in range(GRP):
                    nc.tensor.matmul(pwg[:, gg * BL:(gg + 1) * BL],
                                     whc[:, gg * DIM:(gg + 1) * DIM], pvec[:],
                                     start=True, stop=True)
                t, o0 = jd // 2, (jd % 2) * GRP
                dst = w8a[:, t, 1:NSLOT, o0:o0 + GRP]
                srcv = pwg[:].rearrange("i (g b) -> i b g", g=GRP)
                # alternate copy engines so consecutive stream->conv links
                # ride independent queues (halves the serial chain)
                if jd % 2 == 0:
                    nc.scalar.activation(dst, srcv, AF.Copy,
                                         scale=W_SCALE / WH_SCALE)
                else:
                    nc.vector.tensor_scalar_mul(dst, srcv, W_SCALE / WH_SCALE)

            # ---- resident conv: DoubleRow taps accumulate while Wh streams ----
            RES_PAIRS = [(0, 0), (0, 1), (1, 0), (1, 1), (2, 0), (2, 1)][:NRES_ACTIVE]
            res_box = []

            def open_res():
                pool = tc.tile_pool(name="res", bufs=1, space="PSUM")
                pr = pool.__enter__()
                res_box.append(pool)
                return [pr.tile([128, CW], F32, name=f"pyr{i}")
                        for i in range(NRESC)]

            py_res = []

            def conv_mm(py, b, h2, t, first, last):
                dy, dx = t // 3, t % 3
                # stationary pairs (w_hi const, w_lo sample b): slots {0, 1+b}
                base_w = w8a[:, t, 0, :]
                lhsT = bass.AP(base_w.tensor, base_w.offset,
                               [list(base_w.ap[0]), [(1 + b) * DIM, 2],
                                [1, DIM]])
                base = (h2 * 14 + dy) * HP + dx
                srcs = [xd8]
                if t in EXACT_TAPS:
                    srcs.append(xdl)
                for k, srcbuf in enumerate(srcs):
                    flat = srcbuf[:, b, :, :].rearrange("i r c -> i (r c)")
                    rhs = flat[:, base:base + CW] \
                        .rearrange("i (o m) -> i o m", o=1).broadcast_to([DIM, 2, CW])
                    nc.tensor.matmul(py[0:DIM, :], lhsT, rhs,
                                     start=(first and k == 0),
                                     stop=(last and k == len(srcs) - 1),
                                     perf_mode=DR)

            def emit_res_tap(t):
                for i, (b, h2) in enumerate(RES_PAIRS):
                    conv_mm(py_res[i], b, h2, t, t == 0, t == 8)

            tap_next = 0

            def emit_ready_taps(jc):
                nonlocal tap_next
                while tap_next < 9 and 2 * tap_next + 1 <= jc and py_res:
                    emit_res_tap(tap_next)
                    tap_next += 1

            # stream loop: early chunks drain the remaining xd halves
            # (1-2 per chunk); once xd is done the ps1 psum pool is released
            # to the resident conv accumulators and taps ride the stream.
            jd = 0
            while jd < NCH:
                emit_ph3(jd)
                ndrain = 2 if jd % 2 == 0 else 1
                while ndrain > 0 and xq < len(xd_queue):
                    b2, h22 = xd_queue[xq]
                    emit_xd_half(b2, h22)
                    xq += 1
                    ndrain -= 1
                    if xq == len(xd_queue):
                        ps1.__exit__(None, None, None)
                        py_res.extend(open_res())
                if xq >= len(xd_queue):
                    emit_ready_taps(jd - 2)
                jd += 1
            if xq < len(xd_queue):
                while xq < len(xd_queue):
                    b2, h22 = xd_queue[xq]
                    emit_xd_half(b2, h22)
                    xq += 1
                ps1.__exit__(None, None, None)
                py_res.extend(open_res())
            emit_ready_taps(NCH - 1)

            def emit_gelu(py, b, h2):
                nc.scalar.activation(
                    y97[0:DIM, b, h2 * HF:(h2 + 1) * HF],
                    py[0:DIM, :].rearrange("p (r c) -> p r c", r=14, c=30)[:, :, 0:28],
                    AF.Gelu_apprx_sigmoid, scale=1.0 / W_SCALE)

            for i, (b, h2) in enumerate(RES_PAIRS):
                emit_gelu(py_res[i], b, h2)

            # ---- tail: remaining convs + output projection for all samples ----
            # conv halves and out matmuls share one 6-tile psum ring so
            # write-after-read waits always point several pipeline stages back
            rot = 0
            conv_ring = [0]
            out_ring = [0]

            def next_conv_psum():
                py = py_res[conv_ring[0] % 3]
                conv_ring[0] += 1
                return py

            def next_out_psum():
                py = py_res[3 + out_ring[0] % 3]
                out_ring[0] += 1
                return py

            def emit_out_half(b, ob, h2):
                nonlocal rot
                for c in range(3):
                    po = next_out_psum()
                    nc.tensor.matmul(po[:, 0:HF], w16_sb[0:97, 480 + c * 128:480 + (c + 1) * 128],
                                     y97[:, b, h2 * HF:(h2 + 1) * HF],
                                     start=True, stop=True)
                    dst = ob[:, c, h2, :]
                    if rot % 3 == 2:
                        nc.scalar.activation(dst, po[:, 0:HF], AF.Copy)
                    else:
                        nc.vector.tensor_copy(dst, po[:, 0:HF])
                    rot += 1
                nc.sync.dma_start(yo_d[:, :, b, h2 * HF:(h2 + 1) * HF],
                                  ob[:, :, h2, :])

            def emit_conv_half(b, h2):
                py = next_conv_psum()
                for t in range(9):
                    conv_mm(py, b, h2, t, t == 0, t == 8)
                emit_gelu(py, b, h2)

            conv_halves = [p for p in
                           [(b, h2) for b in range(BL) for h2 in range(2)]
                           if p not in RES_PAIRS]
            out_halves = [(b, h2) for b in range(BL) for h2 in range(2)]
            oq = 0
            obs = {}
            ready = [NRES_ACTIVE + i for i in range(len(conv_halves))]

            def emit_next_out():
                nonlocal oq
                b, h2 = out_halves[oq]
                if h2 == 0:
                    obs[b] = ob_p.tile([128, 3, 2, HF], F16, name=f"ob{b}", tag="ob")
                emit_out_half(b, obs[b], h2)
                oq += 1

            for i, (b, h2) in enumerate(conv_halves):
                emit_conv_half(b, h2)
                want = min(ready[i], len(out_halves))
                while oq < want and oq < (16 * (i + 1)) // len(conv_halves) + 4:
                    emit_next_out()
            while oq < len(out_halves):
                emit_next_out()
            res_box[0].__exit__(None, None, None)
            ps2.__exit__(None, None, None)

    nc.compile()
    return nc


_NC_CACHE = None


def _get_nc():
    global _NC_CACHE
    if _NC_CACHE is None:
        _NC_CACHE = build_nc()
    return _NC_CACHE


def _prep_inputs(x, Wd, bd, Wm1, bm1, Wm2, bm2, Wh, bh, emb, Wu, bu):
    """Host-side prep: pure layout/dtype transforms + x-independent weight
    folding (w0/w_hi/w0r rows, bias packing)."""
    import ml_dtypes
    f16 = np.float16
    f8 = ml_dtypes.float8_e4m3
    Wh = np.asarray(Wh, np.float32)
    w0 = (np.asarray(emb, np.float32) + np.asarray(bm2, np.float32)) @ Wh \
        + np.asarray(bh, np.float32)
    # (e, o, i, t) -> tap-major (e, t, o, i); fp8 with range scaling
    whp = Wh.reshape(E, DIM, DIM, KK * KK).transpose(0, 3, 1, 2).reshape(E, WH_COLS)
    wh8 = np.ascontiguousarray(whp * WH_SCALE).astype(f8)
    # w0 -> [i, (t,o)]; split into fp8 hi + fp16 residual (at W_SCALE)
    w0p = np.ascontiguousarray(
        w0.reshape(DIM, DIM, KK * KK).transpose(2, 0, 1).reshape(TO, DIM).T
    ).astype(np.float32) * W_SCALE
    whi = w0p.astype(f8)
    w0r = (w0p - whi.astype(np.float32))
    w0r_row = (w0r.T.reshape(1, WH_COLS) * (WH_SCALE / W_SCALE)).astype(f8)
    wh8 = np.concatenate([wh8, w0r_row], axis=0)

    w16 = np.zeros((128, 864), f16)
    w16[:, 0:192] = np.asarray(Wm1, np.float32).reshape(3, 128, E) \
        .transpose(1, 0, 2).reshape(128, 192)
    w16[:, 192:480] = np.asarray(Wd, np.float32).reshape(3, 128, DIM) \
        .transpose(1, 0, 2).reshape(128, 288)
    w16[0:DIM, 480:864] = np.asarray(Wu, np.float32)
    w16[DIM:97, 480:864] = np.asarray(bu, np.float32).reshape(1, 384)

    bias = np.zeros((128, 66), np.float32)
    bias[0:E, 0] = np.asarray(bm1, np.float32)
    bias[0:DIM, 1] = np.asarray(bd, np.float32)
    bias[0:E, 2:66] = np.asarray(Wm2, np.float32) / P   # pre-scaled fp32 Wm2

    ones = np.ones((1, BL * P), f16)

    shared = {"w16": w16, "bias": bias, "wh": wh8, "whi": whi,
              "one": ones}
    xs = np.asarray(x, np.float32).astype(f16).reshape(B, P, C)
    in_maps = []
    for k in range(NCORES):
        xt = xs[k * BL:(k + 1) * BL].reshape(BL, P, 3, 128).transpose(3, 2, 0, 1)
        m = dict(shared)
        m["xt"] = np.ascontiguousarray(xt)
        in_maps.append(m)
    return in_maps


def _run(inputs, **spmd_kwargs):
    nc = _get_nc()
    in_maps = _prep_inputs(**inputs)
    res = run_bass_kernel_spmd(nc, in_maps, core_ids=list(range(NCORES)), **spmd_kwargs)
    parts = []
    for r in res.results:
        yo = np.asarray(r["yo"])                      # [128, 3, BL, P] fp16
        parts.append(yo.transpose(2, 3, 1, 0).reshape(BL, P, C))
    out = np.concatenate(parts, 0).astype(np.float32)
    return out.reshape(B, H, W, C), res


def kernel(**inputs) -> np.ndarray:
    out, _ = _run(inputs)
    return out


# revision 57
# speedup vs baseline: 1.0382x; 1.0225x over previous
"""Trainium2 Bass kernel for Convpass-swin hypernet fused adapter.

Reference computation (per batch sample):
  h      = relu(x @ Wm1 + bm1)                    # [B,H,W,64]
  prompt = mean_hw(h) @ Wm2 + bm2                 # [B,64]  (mean commutes with matmul)
  wflat  = (emb + prompt) @ Wh + bh               # [B,96*96*9]
  xd     = quickgelu(x @ Wd + bd)                 # [B,H,W,96]
  y      = quickgelu(conv3x3(xd, wflat))          # per-sample dynamic grouped conv
  out    = y @ Wu + bu                            # [B,H,W,384]

Sharding: data-parallel over batch B=64 across 8 cores (8 samples/core),
weights replicated.

Structure (measured: 62860 ns TimelineSim, rel err 1.68e-2 vs 2e-2 gate;
baseline was 78070 ns):
  * Conv runs in fp8 with DoubleRow matmuls (0.5 cycles/row): the
    stationary pairs (w_hi, w_lo) where w_hi = fp8(S*w0) is a
    host-precomputed constant shared by all samples and
    w_lo = fp8(S*w0 - w_hi + S*delta_b) is the per-sample correction
    produced directly by the hypernet stream's scalar_tensor_tensor.
    w8a layout [i, tap, slot, o] nests slots INSIDE taps: stream writes
    are chunk-local and conv reads tap-local (no false range deps) while
    the DoubleRow pair stride (1+b)*96 stays 16-byte aligned.
    The moving operand is a broadcast (stride-0) pair of one contiguous
    420-element window of the padded fp8 xd buffer (junk columns land in
    unused psum positions, skipped by the strided gelu read).
  * Error budget: taps 7-8 also accumulate the fp8 residual
    xd_lo = fp8(xd16 - xd8) via a second DoubleRow matmul, trimming the
    dominant xd quantization noise; the prompt stays fp16. The whole
    pipeline is deterministic, so the measured error is exact.
  * Engine balance: meta relu+accum on DVE (keeps Act free so the
    px-psum rotation never paces the PE), xd16->xd8 copy and the
    residual subtract on the otherwise-idle GPSIMD, psum->sbuf output
    copies split Act/DVE. The w0 residual seed rides the hypernet
    matmul as a 65th contraction row (ones-row in pvec), so the
    stream's w_lo write is a 1-input scaled copy that ALTERNATES
    between Act and DVE per chunk - consecutive stream->conv
    dependency links ride independent queues, halving the serial
    chain the Tile counting semaphores would otherwise create.
  * bu folds into the output projection as an extra contraction row
    against a constant ones-row in the y tensor, so the 48 psum->sbuf
    copies are pure dtype copies.
  * Schedule: meta+xd interleave at the x-DMA pace; the Wh stream
    starts right after the prompt and is consumed at the chunk-DMA
    pace; the conv+out tail runs on two 3-tile psum rings.
"""
import numpy as np

import concourse.bass as bass
import concourse.tile as tile
import concourse.mybir as mybir
from concourse import bacc
from concourse.bass_utils import run_bass_kernel_spmd

F32 = mybir.dt.float32
F16 = mybir.dt.float16
F8 = mybir.dt.float8e4
AF = mybir.ActivationFunctionType
AX = mybir.AxisListType
DR = mybir.MatmulPerfMode.DoubleRow

# problem constants
B, H, W, C = 64, 28, 28, 384
DIM, E, KK = 96, 64, 3
NCORES = 8
BL = B // NCORES          # samples per core
P = H * W                 # 784 positions per sample
HP = H + 2                # padded spatial
WH_COLS = DIM * DIM * 9   # 82944
NCH = 18                  # Wh stream chunks
CHW = WH_COLS // NCH      # 4608 columns per chunk
GRP = CHW // DIM          # 48 (t,o) groups per chunk
WH_SCALE = 256.0          # fp8 range scaling for Wh
W_SCALE = 64.0            # fp8 scale on conv weights (w_hi/w_lo slots)
HF = P // 2               # 392 positions per half
HPR = HP + 1              # padded rows incl. one spare (window overrun)
CW = 14 * HP              # 420: conv psum window length (junk cols unused)
TO = 9 * DIM              # 864 (t,o) pairs
NSLOT = 1 + BL            # w8 slots: [hi, lo(b=0..7)]
EXACT_TAPS = (8,)         # taps that also accumulate the xd_lo residual
NRESC = 6                 # tail psum ring tiles
NRES_ACTIVE = 0           # resident conv accumulators riding the stream


def build_nc():
    nc = bacc.Bacc("TRN2", target_bir_lowering=False, debug=False)

    xt_d = nc.dram_tensor("xt", [128, 3, BL, P], F16, kind="ExternalInput").ap()
    w16_d = nc.dram_tensor("w16", [128, 864], F16, kind="ExternalInput").ap()
    bias_d = nc.dram_tensor("bias", [128, 66], F32, kind="ExternalInput").ap()
    wh_d = nc.dram_tensor("wh", [E + 1, WH_COLS], F8, kind="ExternalInput").ap()
    whi_d = nc.dram_tensor("whi", [DIM, TO], F8, kind="ExternalInput").ap()
    one_d = nc.dram_tensor("one", [1, BL * P], F16, kind="ExternalInput").ap()
    yo_d = nc.dram_tensor("yo", [128, 3, BL, P], F16, kind="ExternalOutput").ap()

    with tile.TileContext(nc) as tc:
        with (
            tc.tile_pool(name="const", bufs=1) as cp,
            tc.tile_pool(name="persist", bufs=1) as pp,
            tc.tile_pool(name="scr", bufs=3) as sp,
            tc.tile_pool(name="wh", bufs=18) as wh_p,
            tc.tile_pool(name="ob", bufs=6) as ob_p,
        ):
            # ---- constants (meta/xd weights first), then the x stream ----
            w16_sb = cp.tile([128, 864], F16)
            nc.sync.dma_start(w16_sb[:, 0:192], w16_d[:, 0:192])

            xt_sb = pp.tile([128, 3, BL, P], F16)
            # first sample in halves so meta(0,h0) starts as early as possible
            nc.sync.dma_start(xt_sb[:, :, 0, 0:HF], xt_d[:, :, 0, 0:HF])
            nc.sync.dma_start(w16_sb[:, 192:480], w16_d[:, 192:480])
            nc.sync.dma_start(xt_sb[:, :, 0, HF:P], xt_d[:, :, 0, HF:P])
            nc.sync.dma_start(xt_sb[:, :, 1, :], xt_d[:, :, 1, :])
            bias_sb = cp.tile([128, 66], F32)
            nc.sync.dma_start(bias_sb[:], bias_d[:])
            for b in range(2, BL):
                nc.sync.dma_start(xt_sb[:, :, b, :], xt_d[:, :, b, :])
            # w8a: [i, tap, slot, o] (slot 0 = w_hi const). Nesting slots
            # INSIDE taps makes the stream's writes and the conv's reads
            # tap-local (no false deps) while keeping the DoubleRow pair
            # stride (1+b)*96 16-aligned.
            w8a = pp.tile([DIM, 9, NSLOT, DIM], F8)
            nc.sync.dma_start(
                w8a[:, :, 0, :], whi_d[:].rearrange("i (t o) -> i t o", t=9))
            nc.sync.dma_start(w16_sb[:, 480:864], w16_d[:, 480:864])
            # y buffer with a constant ones-row folded into out-proj for bu
            y97 = pp.tile([97, BL, P], F16)
            nc.sync.dma_start(y97[96:97, :, :], one_d[:].rearrange("o (b p) -> o b p", b=BL))

            # ---- persistent state ----
            xd16 = pp.tile([DIM, BL, P], F16)             # fp16 gelu(x@Wd+bd)
            xd8 = pp.tile([DIM, BL, HPR, HP], F8)         # padded fp8 xd
            xdl = pp.tile([DIM, BL, HPR, HP], F8)         # padded fp8 residual
            hsum = pp.tile([E, BL, 2], F32)               # per-(sample,half) relu sums
            hbar = pp.tile([E, BL], F32)
            pvec = pp.tile([E + 1, BL], F16)              # prompt_raw + ones row
            nc.gpsimd.memset(pvec[E:E + 1, :], 1.0)
            zeros = cp.tile([E, 1], F32)
            nc.gpsimd.memset(zeros[:], 0.0)

            # zero the conv halo rings (and window-overrun row) on Pool
            for buf in (xd8, xdl):
                nc.gpsimd.memset(buf[:, :, 0:1, :], 0.0)
                nc.gpsimd.memset(buf[:, :, HP - 1:HPR, :], 0.0)
                nc.gpsimd.memset(buf[:, :, 1:HP - 1, 0:1], 0.0)
                nc.gpsimd.memset(buf[:, :, 1:HP - 1, HP - 1:HP], 0.0)

            ps2 = tc.tile_pool(name="ps2", bufs=1, space="PSUM")
            p2 = ps2.__enter__()

            # ---- phase 1: meta + xd interleaved at the x-DMA pace ----
            ps1 = tc.tile_pool(name="ps1", bufs=1, space="PSUM")
            p1 = ps1.__enter__()

            def emit_meta(b):
                for h2 in range(2):
                    ph = p1.tile([E, HF], F32, name="ph", tag="ph", bufs=2)
                    for c in range(3):
                        nc.tensor.matmul(ph[:], w16_sb[:, c * 64:(c + 1) * 64],
                                         xt_sb[:, c, b, h2 * HF:(h2 + 1) * HF],
                                         start=(c == 0), stop=(c == 2))
                    h_scr = sp.tile([E, HF], F32, tag="hscr")
                    # relu+accum on DVE: keeps Act free to run the xd gelus
                    # ahead of the PE (px-buffer rotation would otherwise
                    # pace the whole stream at Act speed)
                    nc.vector.scalar_tensor_tensor(
                        h_scr[:], ph[:], bias_sb[0:E, 0:1],
                        zeros[:].broadcast_to([E, HF]),
                        op0=mybir.AluOpType.add,
                        op1=mybir.AluOpType.max,
                        accum_out=hsum[:, b, h2:h2 + 1])

            def emit_xd_half(b, h2):
                px = p1.tile([DIM, HF], F32, name="px", tag="px", bufs=3)
                for c in range(3):
                    nc.tensor.matmul(px[:], w16_sb[:, 192 + c * 96:192 + (c + 1) * 96],
                                     xt_sb[:, c, b, h2 * HF:(h2 + 1) * HF],
                                     start=(c == 0), stop=(c == 2))
                nc.scalar.activation(
                    xd16[:, b, h2 * HF:(h2 + 1) * HF], px[:],
                    AF.Gelu_apprx_sigmoid, bias=bias_sb[0:DIM, 1:2])
                x16v = xd16[:, b, h2 * HF:(h2 + 1) * HF] \
                    .rearrange("p (r c) -> p r c", r=14)
                d8 = xd8[:, b, 1 + h2 * 14:15 + h2 * 14, 1:29]
                nc.gpsimd.tensor_copy(d8, x16v)
                # residual: xd16 - xd8 (on the otherwise-idle GPSIMD)
                nc.gpsimd.tensor_sub(
                    xdl[:, b, 1 + h2 * 14:15 + h2 * 14, 1:29], x16v, d8)

            # meta is x-DMA-paced; fill the PE slack with early samples' xd
            xd_queue = [(b, h2) for b in range(BL) for h2 in range(2)]
            xq = 0
            for b in range(BL):
                emit_meta(b)
                # one xd half per sample keeps PE just under the DMA pace
                if b >= 1 and xq < len(xd_queue):
                    xb, xh = xd_queue[xq]
                    if xb < b:   # its x is already resident
                        emit_xd_half(xb, xh)
                        xq += 1
            # prompt chain
            nc.vector.reduce_sum(hbar[:], hsum[:], axis=AX.X)
            ppm = p1.tile([E, BL], F32, name="ppm", tag="ppm", bufs=1)
            nc.tensor.matmul(ppm[:], bias_sb[0:E, 2:66], hbar[:],
                             start=True, stop=True)
            nc.scalar.activation(pvec[0:E, :], ppm[:], AF.Copy)

            # ---- hypernet stream: w_lo slots = fp8(w0r + delta*W_SCALE) ----
            def emit_ph3(jd):
                whc = wh_p.tile([E + 1, CHW], F8, tag="whc")
                nc.sync.dma_start(whc[:], wh_d[:, jd * CHW:(jd + 1) * CHW])
                pwg = p2.tile([DIM, GRP * BL], F32, name="pwg", tag="pwg", bufs=2)
                for gg in range(GRP):
                    nc.tensor.matmul(pwg[:, gg * BL:(gg + 1) * BL],
                                     whc[:, gg * DIM:(gg + 1) * DIM], pvec[:],
                                     start=True, stop=True)
                t, o0 = jd // 2, (jd % 2) * GRP
                dst = w8a[:, t, 1:NSLOT, o0:o0 + GRP]
                srcv = pwg[:].rearrange("i (g b) -> i b g", g=GRP)
                # alternate copy engines so consecutive stream->conv links
                # ride independent queues (halves the serial chain)
                if jd % 2 == 0:
                    nc.scalar.activation(dst, srcv, AF.Copy,
                                         scale=W_SCALE / WH_SCALE)
                else:
                    nc.vector.tensor_scalar_mul(dst, srcv, W_SCALE / WH_SCALE)

            # ---- resident conv: DoubleRow taps accumulate while Wh streams ----
            RES_PAIRS = [(0, 0), (0, 1), (1, 0), (1, 1), (2, 0), (2, 1)][:NRES_ACTIVE]
            res_box = []

            def open_res():
                pool = tc.tile_pool(name="res", bufs=1, space="PSUM")
                pr = pool.__enter__()
                res_box.append(pool)
                return [pr.tile([128, CW], F32, name=f"pyr{i}")
                        for i in range(NRESC)]

            py_res = []

            def conv_mm(py, b, h2, t, first, last):
                dy, dx = t // 3, t % 3
                # stationary pairs (w_hi const, w_lo sample b): slots {0, 1+b}
                base_w = w8a[:, t, 0, :]
                lhsT = bass.AP(base_w.tensor, base_w.offset,
                               [list(base_w.ap[0]), [(1 + b) * DIM, 2],
                                [1, DIM]])
                base = (h2 * 14 + dy) * HP + dx
                srcs = [xd8]
                if t in EXACT_TAPS:
                    srcs.append(xdl)
                for k, srcbuf in enumerate(srcs):
                    flat = srcbuf[:, b, :, :].rearrange("i r c -> i (r c)")
                    rhs = flat[:, base:base + CW] \
                        .rearrange("i (o m) -> i o m", o=1).broadcast_to([DIM, 2, CW])
                    nc.tensor.matmul(py[0:DIM, :], lhsT, rhs,
                                     start=(first and k == 0),
                                     stop=(last and k == len(srcs) - 1),
                                     perf_mode=DR)

            def emit_res_tap(t):
                for i, (b, h2) in enumerate(RES_PAIRS):
                    conv_mm(py_res[i], b, h2, t, t == 0, t == 8)

            tap_next = 0

            def emit_ready_taps(jc):
                nonlocal tap_next
                while tap_next < 9 and 2 * tap_next + 1 <= jc and py_res:
                    emit_res_tap(tap_next)
                    tap_next += 1

            # stream loop: early chunks drain the remaining xd halves
            # (1-2 per chunk); once xd is done the ps1 psum pool is released
            # to the resident conv accumulators and taps ride the stream.
            jd = 0
            while jd < NCH:
                emit_ph3(jd)
                ndrain = 2 if jd % 2 == 0 else 1
                while ndrain > 0 and xq < len(xd_queue):
                    b2, h22 = xd_queue[xq]
                    emit_xd_half(b2, h22)
                    xq += 1
                    ndrain -= 1
                    if xq == len(xd_queue):
                        ps1.__exit__(None, None, None)
                        py_res.extend(open_res())
                if xq >= len(xd_queue):
                    emit_ready_taps(jd - 2)
                jd += 1
            if xq < len(xd_queue):
                while xq < len(xd_queue):
                    b2, h22 = xd_queue[xq]
                    emit_xd_half(b2, h22)
                    xq += 1
                ps1.__exit__(None, None, None)
                py_res.extend(open_res())
            emit_ready_taps(NCH - 1)

            def emit_gelu(py, b, h2):
                nc.scalar.activation(
                    y97[0:DIM, b, h2 * HF:(h2 + 1) * HF],
                    py[0:DIM, :].rearrange("p (r c) -> p r c", r=14, c=30)[:, :, 0:28],
                    AF.Gelu_apprx_sigmoid, scale=1.0 / W_SCALE)

            for i, (b, h2) in enumerate(RES_PAIRS):
                emit_gelu(py_res[i], b, h2)

            # ---- tail: remaining convs + output projection for all samples ----
            # conv halves and out matmuls share one 6-tile psum ring so
            # write-after-read waits always point several pipeline stages back
            rot = 0
            conv_ring = [0]
            out_ring = [0]

            def next_conv_psum():
                py = py_res[conv_ring[0] % 3]
                conv_ring[0] += 1
                return py

            def next_out_psum():
                py = py_res[3 + out_ring[0] % 3]
                out_ring[0] += 1
                return py

            def emit_out_half(b, ob, h2):
                nonlocal rot
                for c in range(3):
                    po = next_out_psum()
                    nc.tensor.matmul(po[:, 0:HF], w16_sb[0:97, 480 + c * 128:480 + (c + 1) * 128],
                                     y97[:, b, h2 * HF:(h2 + 1) * HF],
                                     start=True, stop=True)
                    dst = ob[:, c, h2, :]
                    if rot % 3 == 2:
                        nc.scalar.activation(dst, po[:, 0:HF], AF.Copy)
                    else:
                        nc.vector.tensor_copy(dst, po[:, 0:HF])
                    rot += 1
                nc.sync.dma_start(yo_d[:, :, b, h2 * HF:(h2 + 1) * HF],
                                  ob[:, :, h2, :])

            def emit_conv_half(b, h2):
                py = next_conv_psum()
                for t in range(9):
                    conv_mm(py, b, h2, t, t == 0, t == 8)
                emit_gelu(py, b, h2)

            conv_halves = [p for p in
                           [(b, h2) for b in range(BL) for h2 in range(2)]
                           if p not in RES_PAIRS]
            out_halves = [(b, h2) for b in range(BL) for h2 in range(2)]
            oq = 0
            obs = {}
            ready = [NRES_ACTIVE + i for i in range(len(conv_halves))]

            def emit_next_out():
                nonlocal oq
                b, h2 = out_halves[oq]
                if h2 == 0:
                    obs[b] = ob_p.tile([128, 3, 2, HF], F16, name=f"ob{b}", tag="ob")
                emit_out_half(b, obs[b], h2)
                oq += 1

            for i, (b, h2) in enumerate(conv_halves):
                emit_conv_half(b, h2)
                want = min(ready[i], len(out_halves))
                while oq < want and oq < (16 * (i + 1)) // len(conv_halves) + 4:
                    emit_next_out()
            while oq < len(out_halves):
                emit_next_out()
            res_box[0].__exit__(None, None, None)
            ps2.__exit__(None, None, None)

    nc.compile()
    return nc


_NC_CACHE = None


def _get_nc():
    global _NC_CACHE
    if _NC_CACHE is None:
        _NC_CACHE = build_nc()
    return _NC_CACHE


def _prep_inputs(x, Wd, bd, Wm1, bm1, Wm2, bm2, Wh, bh, emb, Wu, bu):
    """Host-side prep: pure layout/dtype transforms + x-independent weight
    folding (w0/w_hi/w0r rows, bias packing)."""
    import ml_dtypes
    f16 = np.float16
    f8 = ml_dtypes.float8_e4m3
    Wh = np.asarray(Wh, np.float32)
    w0 = (np.asarray(emb, np.float32) + np.asarray(bm2, np.float32)) @ Wh \
        + np.asarray(bh, np.float32)
    # (e, o, i, t) -> tap-major (e, t, o, i); fp8 with range scaling
    whp = Wh.reshape(E, DIM, DIM, KK * KK).transpose(0, 3, 1, 2).reshape(E, WH_COLS)
    wh8 = np.ascontiguousarray(whp * WH_SCALE).astype(f8)
    # w0 -> [i, (t,o)]; split into fp8 hi + fp16 residual (at W_SCALE)
    w0p = np.ascontiguousarray(
        w0.reshape(DIM, DIM, KK * KK).transpose(2, 0, 1).reshape(TO, DIM).T
    ).astype(np.float32) * W_SCALE
    whi = w0p.astype(f8)
    w0r = (w0p - whi.astype(np.float32))
    w0r_row = (w0r.T.reshape(1, WH_COLS) * (WH_SCALE / W_SCALE)).astype(f8)
    wh8 = np.concatenate([wh8, w0r_row], axis=0)

    w16 = np.zeros((128, 864), f16)
    w16[:, 0:192] = np.asarray(Wm1, np.float32).reshape(3, 128, E) \
        .transpose(1, 0, 2).reshape(128, 192)
    w16[:, 192:480] = np.asarray(Wd, np.float32).reshape(3, 128, DIM) \
        .transpose(1, 0, 2).reshape(128, 288)
    w16[0:DIM, 480:864] = np.asarray(Wu, np.float32)
    w16[DIM:97, 480:864] = np.asarray(bu, np.float32).reshape(1, 384)

    bias = np.zeros((128, 66), np.float32)
    bias[0:E, 0] = np.asarray(bm1, np.float32)
    bias[0:DIM, 1] = np.asarray(bd, np.float32)
    bias[0:E, 2:66] = np.asarray(Wm2, np.float32) / P   # pre-scaled fp32 Wm2

    ones = np.ones((1, BL * P), f16)

    shared = {"w16": w16, "bias": bias, "wh": wh8, "whi": whi,
              "one": ones}
    xs = np.asarray(x, np.float32).astype(f16).reshape(B, P, C)
    in_maps = []
    for k in range(NCORES):
        xt = xs[k * BL:(k + 1) * BL].reshape(BL, P, 3, 128).transpose(3, 2, 0, 1)
        m = dict(shared)
        m["xt"] = np.ascontiguousarray(xt)
        in_maps.append(m)
    return in_maps


def _run(inputs, **spmd_kwargs):
    nc = _get_nc()
    in_maps = _prep_inputs(**inputs)
    res = run_bass_kernel_spmd(nc, in_maps, core_ids=list(range(NCORES)), **spmd_kwargs)
    parts = []
    for r in res.results:
        yo = np.asarray(r["yo"])                      # [128, 3, BL, P] fp16
        parts.append(yo.transpose(2, 3, 1, 0).reshape(BL, P, C))
    out = np.concatenate(parts, 0).astype(np.float32)
    return out.reshape(B, H, W, C), res


def kernel(**inputs) -> np.ndarray:
    out, _ = _run(inputs)
    return out
